# revision 4
# baseline (speedup 1.0000x reference)
# Trainium2 Bass kernel for nn_DecoderMHA (dense decoder multi-head attention).
#
# Sharding (8 NeuronCores): batch (4) x tensor-parallel over heads (2).
# Core c handles batch b = c//2 and heads [tp*8, tp*8+8) where tp = c%2,
# i.e. a 512-wide slice of the QKV projection output dim and the matching
# 512 rows of Wo^T. Per-core partial outputs are summed on the host
# (y[b] = part[b,0] + part[b,1] + bo).
#
# Per-core pipeline (matmul operands bf16, fp32 PSUM accumulation):
#   A) Q^T/K^T [512,2048] and V [2048,512] projections from x^T.
#   B) Per head: transposed-scores attention.  scores^T[k,q] tiles land in
#      PSUM, ScalarE applies exp(0.125*s + pad_bias) into SBUF, causal mask
#      multiply on diagonal blocks only, then attn@V accumulates out^T via
#      a v-augmented-with-ones lhsT so row 64 of PSUM is the softmax
#      denominator.  Normalisation: denominators round-trip through DRAM
#      into a [128,8] layout for a cheap batched reciprocal, then a second
#      round-trip broadcasts 1/den across partitions for one multiply.
#   C) y = out @ Wo^T partial via out^T-as-lhsT matmuls.
import os
import numpy as np

BSZ, SEQ, DM = 4, 2048, 1024
HEADS, DK = 16, 64
NCORES, TP = 8, 2
E = DM // TP          # 512 per-core projection slice
HPC = HEADS // TP     # 8 heads per core
P = 128
NDC = DM // P         # 8 contraction chunks
NEC = E // P          # 4 e-chunks
NSC = SEQ // P        # 16 sequence chunks
NSB = SEQ // 512      # 4 sequence blocks
SCALE = 1.0 / float(np.sqrt(DK))

_CACHED = {}


def _split_sync_waits(nc, mybir, max_waits=1):
    """The walrus in this container only accepts one sync-wait per
    instruction; move excess waits onto NoOps in front."""
    n = 0
    for fn in nc.m.functions:
        for bb in fn.blocks:
            insts = bb.instructions
            i = 0
            while i < len(insts):
                inst = insts[i]
                si = getattr(inst, "sync_info", None)
                if si is not None and si.on_wait and len(si.on_wait) > max_waits:
                    waits = list(si.on_wait)
                    extra, keep = waits[:-max_waits], waits[-max_waits:]
                    si.on_wait = keep
                    pos = i
                    for j in range(0, len(extra), max_waits):
                        nop = mybir.InstNoOp(
                            name=nc.get_next_instruction_name(),
                            sync_info=mybir.SyncInfo(
                                on_wait=extra[j:j + max_waits], on_update=[]),
                            bass_nofuse=True,
                            engine=inst.engine,
                        )
                        insts.insert(pos, nop)
                        pos += 1
                        i += 1
                        n += 1
                i += 1
    return n


def _build():
    import concourse.bass as bass
    from concourse import mybir
    from concourse.tile import TileContext

    f32 = mybir.dt.float32
    bf16 = mybir.dt.bfloat16
    Exp = mybir.ActivationFunctionType.Exp
    MUL = mybir.AluOpType.mult
    ADD = mybir.AluOpType.add

    nc = bass.Bass("TRN2", target_bir_lowering=False, debug=False,
                   num_devices=NCORES)

    # DRAM I/O (per-core layouts, pre-tiled on host)
    xt = nc.dram_tensor("xt", [P, NDC, SEQ], bf16, kind="ExternalInput")
    wq = nc.dram_tensor("wq", [P, NDC, E], bf16, kind="ExternalInput")
    wk = nc.dram_tensor("wk", [P, NDC, E], bf16, kind="ExternalInput")
    wv = nc.dram_tensor("wv", [P, NDC, E], bf16, kind="ExternalInput")
    wo = nc.dram_tensor("wo", [P, NEC, DM], bf16, kind="ExternalInput")
    bqt = nc.dram_tensor("bqt", [P, NEC], f32, kind="ExternalInput")
    bkt = nc.dram_tensor("bkt", [P, NEC], f32, kind="ExternalInput")
    bvb = nc.dram_tensor("bvb", [P, E], f32, kind="ExternalInput")
    cm = nc.dram_tensor("cm", [P, P], f32, kind="ExternalInput")
    pb = nc.dram_tensor("pb", [P, NSC], f32, kind="ExternalInput")
    ones = nc.dram_tensor("ones", [P, HPC], bf16, kind="ExternalInput")
    y = nc.dram_tensor("y", [SEQ, DM], f32, kind="ExternalOutput")

    with TileContext(nc) as tc:
        with (
            tc.tile_pool(name="persist", bufs=1) as pp,
            tc.tile_pool(name="psS", bufs=2, space="PSUM") as psS,
            tc.tile_pool(name="psO", bufs=2, space="PSUM") as psO,
            tc.tile_pool(name="scr", bufs=4, space="DRAM") as scr,
        ):
            # ---- persistent SBUF ----
            qT = [pp.tile([P, SEQ], bf16, tag=f"qT{t}", name=f"qT{t}")
                  for t in range(NEC)]
            kT = [pp.tile([P, SEQ], bf16, tag=f"kT{t}", name=f"kT{t}")
                  for t in range(NEC)]
            vA = [pp.tile([P, HPC, DK + 1], bf16, tag=f"vA{g}", name=f"vA{g}")
                  for g in range(NSC)]
            cm_s = pp.tile([P, P], f32, tag="cm")
            pb_s = pp.tile([P, NSC], f32, tag="pb")
            bq_s = pp.tile([P, NEC], f32, tag="bq")
            bk_s = pp.tile([P, NEC], f32, tag="bk")
            bv_s = pp.tile([P, E], f32, tag="bv")
            nc.sync.dma_start(cm_s[:], cm[:])
            nc.sync.dma_start(pb_s[:], pb[:])
            nc.sync.dma_start(bq_s[:], bqt[:])
            nc.sync.dma_start(bk_s[:], bkt[:])
            nc.sync.dma_start(bv_s[:], bvb[:])
            for g in range(NSC):
                nc.sync.dma_start(vA[g][:, :, DK:DK + 1], ones[:, :])

            # =========== Phase A: QKV projections ===========
            with tc.tile_pool(name="stageA", bufs=1) as pa:
                wq_s = pa.tile([P, NDC, E], bf16, tag="wq")
                wk_s = pa.tile([P, NDC, E], bf16, tag="wk")
                wv_s = pa.tile([P, NDC, E], bf16, tag="wv")
                nc.sync.dma_start(wq_s[:], wq[:])
                nc.sync.dma_start(wk_s[:], wk[:])
                nc.sync.dma_start(wv_s[:], wv[:])
                for sb in range(NSB):
                    xt_s = pa.tile([P, NDC, 512], bf16, tag="xt", bufs=2)
                    nc.sync.dma_start(xt_s[:], xt[:, :, sb * 512:(sb + 1) * 512])
                    # Q^T, K^T: psum[e-chunk, s-block]
                    for (w_s, b_s, dst) in ((wq_s, bq_s, qT), (wk_s, bk_s, kT)):
                        for ec in range(NEC):
                            psum = psS.tile([P, 512], f32, tag="strip",
                                            name=f"pj{sb}_{ec}")
                            for dc in range(NDC):
                                nc.tensor.matmul(
                                    psum[:],
                                    w_s[:, dc, ec * P:(ec + 1) * P],
                                    xt_s[:, dc, :],
                                    start=(dc == 0), stop=(dc == NDC - 1))
                            nc.vector.tensor_tensor(
                                dst[ec][:, sb * 512:(sb + 1) * 512],
                                psum[:],
                                b_s[:, ec:ec + 1].to_broadcast([P, 512]),
                                ADD)
                    # V natural [s, e]
                    for ssc in range(4):
                        g = sb * 4 + ssc
                        psum = psS.tile([P, 512], f32, tag="strip",
                                        name=f"pv{g}")
                        for dc in range(NDC):
                            nc.tensor.matmul(
                                psum[:],
                                xt_s[:, dc, ssc * P:(ssc + 1) * P],
                                wv_s[:, dc, :],
                                start=(dc == 0), stop=(dc == NDC - 1))
                        nc.vector.tensor_tensor(
                            vA[g][:, :, 0:DK],
                            psum[:].rearrange("p (h d) -> p h d", h=HPC),
                            bv_s[:].rearrange("p (h d) -> p h d", h=HPC),
                            ADD)

            # =========== Phase B + C pools ===========
            with tc.tile_pool(name="stageB", bufs=1) as pbp:
                outT = [pbp.tile([P, SEQ], bf16, tag=f"oT{t}", name=f"oT{t}")
                        for t in range(NEC)]

                for h in range(HPC):
                    t, r0 = h // 2, 64 * (h % 2)
                    for qh in range(2):
                        qh0 = qh * 1024
                        nkc = (qh0 + 1024) // P  # 8 or 16 k-chunks
                        stop0 = (qh0 + 512) // P - 1   # last kc hitting block 0
                        stop1 = nkc - 1
                        ops_t = psO.tile([DK + 1, 2, 512], f32, tag="ops",
                                         name=f"op{h}_{qh}")
                        for kc in range(nkc):
                            k0 = kc * P
                            off = max(0, k0 - qh0)
                            strip = psS.tile([P, 1024], f32, tag="strip",
                                             name=f"st{h}_{qh}_{kc}")
                            lh_k = kT[t][r0:r0 + DK, k0:k0 + P]
                            if off < 512:
                                nc.tensor.matmul(
                                    strip[:, off:512], lh_k,
                                    qT[t][r0:r0 + DK, qh0 + off:qh0 + 512],
                                    start=True, stop=True)
                            o2 = max(off, 512)
                            nc.tensor.matmul(
                                strip[:, o2:1024], lh_k,
                                qT[t][r0:r0 + DK, qh0 + o2:qh0 + 1024],
                                start=True, stop=True)
                            exp_s = pbp.tile([P, 1024], bf16, tag="exp",
                                             bufs=3, name=f"ex{h}_{qh}_{kc}")
                            nc.scalar.activation(
                                exp_s[:, off:1024], strip[:, off:1024], Exp,
                                bias=pb_s[:, kc:kc + 1], scale=SCALE)
                            if k0 >= qh0:
                                nc.vector.tensor_tensor(
                                    exp_s[:, off:off + P],
                                    exp_s[:, off:off + P], cm_s[:], MUL)
                            lh_v = vA[kc][:, h, :]
                            if off < 512:
                                nc.tensor.matmul(
                                    ops_t[:, 0, off:512], lh_v,
                                    exp_s[:, off:512],
                                    start=(kc == 0), stop=(kc == stop0))
                            nc.tensor.matmul(
                                ops_t[:, 1, o2 - 512:512], lh_v,
                                exp_s[:, o2:1024],
                                start=(kc == 0), stop=(kc == stop1))
                        # normalise: batched reciprocal via DRAM round-trips
                        den_s = pbp.tile([1, 1024], f32, tag="den", bufs=3,
                                         name=f"dn{h}_{qh}")
                        nc.vector.tensor_copy(
                            den_s[:], ops_t[DK:DK + 1, :, :])
                        sc1 = scr.tile([1, 1024], f32, tag="scr1",
                                       name=f"sc1_{h}_{qh}")
                        nc.sync.dma_start(sc1[:], den_s[:])
                        den_t = pbp.tile([P, HPC], f32, tag="dent", bufs=3,
                                         name=f"dt{h}_{qh}")
                        nc.sync.dma_start(
                            den_t[:],
                            sc1[0, :].rearrange("(p j) -> p j", p=P))
                        rcp_t = pbp.tile([P, HPC], f32, tag="rcpt", bufs=3,
                                         name=f"rt{h}_{qh}")
                        nc.vector.reciprocal(rcp_t[:], den_t[:])
                        sc2 = scr.tile([1, 1024], f32, tag="scr2",
                                       name=f"sc2_{h}_{qh}")
                        nc.sync.dma_start(
                            sc2[0, :].rearrange("(p j) -> p j", p=P),
                            rcp_t[:])
                        for b in range(2):
                            bc = pbp.tile([DK, 512], f32, tag="bc", bufs=3,
                                          name=f"bc{h}_{qh}_{b}")
                            nc.sync.dma_start(
                                bc[:],
                                sc2[0:1, b * 512:(b + 1) * 512]
                                .to_broadcast([DK, 512]))
                            nc.vector.tensor_tensor(
                                outT[t][r0:r0 + DK,
                                        qh0 + b * 512:qh0 + (b + 1) * 512],
                                ops_t[0:DK, b, :], bc[:], MUL)

                # =========== Phase C: output projection ===========
                with tc.tile_pool(name="stageC", bufs=1) as pc:
                    wo_s = pc.tile([P, NEC, DM], bf16, tag="wo")
                    nc.sync.dma_start(wo_s[:], wo[:])
                    for sc in range(NSC):
                        y_s = pc.tile([P, DM], f32, tag="ys", bufs=2)
                        for eh in range(2):
                            psum = psS.tile([P, 512], f32, tag="strip",
                                            name=f"py{sc}_{eh}")
                            for dcc in range(NEC):
                                nc.tensor.matmul(
                                    psum[:],
                                    outT[dcc][:, sc * P:(sc + 1) * P],
                                    wo_s[:, dcc, eh * 512:(eh + 1) * 512],
                                    start=(dcc == 0), stop=(dcc == NEC - 1))
                            nc.vector.tensor_copy(
                                y_s[:, eh * 512:(eh + 1) * 512], psum[:])
                        nc.sync.dma_start(y[sc * P:(sc + 1) * P, :], y_s[:])

    _split_sync_waits(nc, mybir)
    return nc


def _prep_inputs(x, pad_mask, Wq, bq, Wk, bk, Wv, bv, Wo, bo):
    """Build the 8 per-core input maps."""
    import ml_dtypes
    bf16 = ml_dtypes.bfloat16

    def tile3(a, n):  # [n*128, F] -> [128, n, F] in bf16
        return np.ascontiguousarray(
            a.reshape(n, P, a.shape[1]).transpose(1, 0, 2).astype(bf16))

    cmv = (np.arange(P)[:, None] <= np.arange(P)[None, :]).astype(np.float32)
    in_maps = []
    for c in range(NCORES):
        b, tp = c // 2, c % 2
        sl = slice(tp * E, (tp + 1) * E)
        xT = np.ascontiguousarray(x[b].T.astype(np.float32))
        padb = np.where(pad_mask[b, 0, 0] == 1, -1e30, 0.0).astype(np.float32)
        in_maps.append({
            "xt": tile3(xT, NDC),
            "wq": tile3(np.ascontiguousarray(Wq.T[:, sl]), NDC),
            "wk": tile3(np.ascontiguousarray(Wk.T[:, sl]), NDC),
            "wv": tile3(np.ascontiguousarray(Wv.T[:, sl]), NDC),
            "wo": tile3(np.ascontiguousarray(Wo.T[sl, :]), NEC),
            "bqt": np.ascontiguousarray(bq[sl].reshape(NEC, P).T),
            "bkt": np.ascontiguousarray(bk[sl].reshape(NEC, P).T),
            "bvb": np.ascontiguousarray(np.tile(bv[sl][None, :], (P, 1))),
            "cm": cmv,
            "ones": np.ones((P, HPC), dtype=bf16),
            "pb": np.ascontiguousarray(padb.reshape(NSC, P).T),
        })
    return in_maps


def _enable_tracing():
    """Register the NTFF profile hook (the image lacks antenv.axon_hooks)
    and neuter the bucket upload the trace path attempts."""
    import sys
    import types
    try:
        import antenv.axon_hooks  # noqa: F401
    except ImportError:
        from trn_agent_boot.trn_boot import _ntff_profile_via_ctypes
        m = types.ModuleType("antenv.axon_hooks")
        hook = _ntff_profile_via_ctypes("/opt/axon/libaxon_pjrt.so")
        m.get_axon_ntff_profile_hook = lambda: hook
        sys.modules["antenv.axon_hooks"] = m
    import concourse.bass_utils as bu
    bu.upload_artifacts = lambda tmpdir: tmpdir


def kernel_with_stats(inputs, trace=False):
    from concourse.bass_utils import run_bass_kernel_spmd

    if trace:
        try:
            _enable_tracing()
        except Exception:
            trace = False

    if "nc" not in _CACHED:
        _CACHED["nc"] = _build()
    nc = _CACHED["nc"]
    in_maps = _prep_inputs(**inputs)
    res = run_bass_kernel_spmd(nc, in_maps, core_ids=list(range(NCORES)),
                               trace=trace)
    bo = inputs["bo"].astype(np.float32)
    out = np.empty((BSZ, SEQ, DM), dtype=np.float32)
    for b in range(BSZ):
        out[b] = res.results[2 * b]["y"] + res.results[2 * b + 1]["y"] + bo
    return out, res


def kernel(**inputs):
    out, _ = kernel_with_stats(
        inputs, trace=bool(int(os.environ.get("KERNEL_TRACE", "0"))))
    return out
